# revision 22
# baseline (speedup 1.0000x reference)
"""Trainium2 Bass kernel for nn_MitoticTransformerBlock (full causal attention +
soft-gated 2-expert FFN), sharded over 8 NeuronCores.

Sharding: attention by heads (2 heads/core), experts tensor-parallel over the ff
dim (512/core/expert).  Each core folds x/8 into its out-proj partial, so the
chunked AllReduce directly yields x2 = x + attn on every core; expert partials
(+x2/8) are ReduceScattered in bf16 (transposed layout, two halves per chunk)
so each core ends up with a 64-row d-slice of every token's final output.

v4: one 4-slot [128,1024] PSUM pool (PV accumulator included), paired matmul
groups (two 512-wide groups per PSUM tile) halving ACT/DVE op counts,
readiness-ordered emission with FFN prologue/body split and double-buffered
x2T, split ReduceScatter to shrink the exposed tail.
"""

import sys

sys.path.insert(0, "/opt/trn_rl_repo")

import numpy as np
import ml_dtypes

import concourse.bass as bass
import concourse.tile as tile
import concourse.mybir as mybir
from concourse import bacc
from concourse.bass_utils import run_bass_kernel_spmd

F32 = mybir.dt.float32
BF16 = mybir.dt.bfloat16
AF = mybir.ActivationFunctionType
OP = mybir.AluOpType
NPBF16 = ml_dtypes.bfloat16

NCORES = 8
B, T, D, H, FF = 1, 4096, 1024, 16, 4096
HD = D // H          # 64
DB = D // 128        # 8 d-blocks
NTQ = T // 512       # 8 attention q-chunks of 512
NKT = T // 128       # 32 key tiles
TC = 1024            # AllReduce/FFN chunk (tokens)
NCH = T // TC        # 4 chunks
FFS = FF // NCORES   # 512 ff slice per core per expert
NFB = FFS // 128     # 4 ff blocks
LN_EPS = 1e-5
VW = HD + 1          # 65: v columns + ones column
TH2 = T // 2
NKH = NKT // 2

_COMPILED = None


def _build_nc():
    nc = bacc.Bacc("TRN2", target_bir_lowering=False, debug=False,
                   num_devices=NCORES)

    def din(name, shape, dt):
        return nc.dram_tensor(name, shape, dt, kind="ExternalInput").ap()

    xT = din("xT", [D, T], BF16)
    wq = din("wq", [128, DB, 128], BF16)
    wk = din("wk", [128, DB, 128], BF16)
    wv = din("wv", [128, DB, 128], BF16)
    bq = din("bq", [128, 1], F32)
    bk = din("bk", [128, 1], F32)
    bv = din("bv", [1, 128], BF16)
    wo = din("wo", [128, DB, 128], BF16)
    ln2su = din("ln2su", [128, DB, 2], BF16)
    ln2do = din("ln2do", [128, DB, 2], BF16)
    scn = din("scn", [2, 1], F32)
    cbias = din("cbias", [128, 2], F32)
    masks = din("masks", [128, 4, 512], BF16)
    sel2b = din("sel2b", [2, 2, 64], BF16)
    sele = din("sele", [2, 2, 128], BF16)
    egt = din("egt", [2, 128, DB, NFB, 128], BF16)
    ept = din("ept", [2, 128, DB, NFB, 128], BF16)
    eot = din("eot", [2, 128, NFB, D], BF16)

    out_rows = nc.dram_tensor("out_rows", [NCH, 2, 64, TC], BF16,
                              kind="ExternalOutput").ap()

    with tile.TileContext(nc) as tc:
        const = tc.alloc_tile_pool(name="const", bufs=1)
        work2 = tc.alloc_tile_pool(name="work2", bufs=2)
        work4 = tc.alloc_tile_pool(name="work4", bufs=4)
        chunk1 = tc.alloc_tile_pool(name="chunk1", bufs=1)
        chunk2 = tc.alloc_tile_pool(name="chunk2", bufs=2)
        psB = tc.alloc_tile_pool(name="psB", bufs=4, space="PSUM")
        dram = tc.alloc_tile_pool(name="dram", bufs=1, space="DRAM")

        dma = nc.sync.dma_start

        def pst(name):
            return psB.tile([128, 1024], F32, tag="b", name=name)

        # ---- setup: constants into SBUF ----
        wq_sb = const.tile([128, DB, 128], BF16); dma(out=wq_sb, in_=wq)
        wk_sb = const.tile([128, DB, 128], BF16); dma(out=wk_sb, in_=wk)
        wv_sb = const.tile([128, DB, 128], BF16); dma(out=wv_sb, in_=wv)
        bq_sb = const.tile([128, 1], F32); dma(out=bq_sb, in_=bq)
        bk_sb = const.tile([128, 1], F32); dma(out=bk_sb, in_=bk)
        bv_sb = const.tile([1, 128], BF16); dma(out=bv_sb, in_=bv)
        wo_sb = const.tile([128, DB, 128], BF16); dma(out=wo_sb, in_=wo)
        ln2su_sb = const.tile([128, DB, 2], BF16); dma(out=ln2su_sb, in_=ln2su)
        ln2do_sb = const.tile([128, DB, 2], BF16); dma(out=ln2do_sb, in_=ln2do)
        scn_sb = const.tile([2, 1], F32); dma(out=scn_sb, in_=scn)
        cb_sb = const.tile([128, 2], F32); dma(out=cb_sb, in_=cbias)
        mask_sb = const.tile([128, 4, 512], BF16); dma(out=mask_sb, in_=masks)
        sel2_sb = const.tile([2, 2, 64], BF16); dma(out=sel2_sb, in_=sel2b)
        sele_sb = const.tile([2, 2, 128], BF16); dma(out=sele_sb, in_=sele)
        ones128 = const.tile([128, 128], BF16)
        nc.gpsimd.memset(ones128, 1.0)
        ones1 = const.tile([1, 128], BF16)
        nc.gpsimd.memset(ones1, 1.0)
        eps2 = const.tile([2, 1], F32)
        nc.gpsimd.memset(eps2, LN_EPS)
        eps128 = const.tile([128, 1], F32)
        nc.gpsimd.memset(eps128, LN_EPS)

        q_h = [const.tile([128, TH2], BF16, name=f"q{i}") for i in range(2)]
        k_h = [const.tile([128, TH2], BF16, name=f"k{i}") for i in range(2)]
        v_h = [const.tile([128, NKH * 2 * VW], BF16, name=f"v{i}")
               for i in range(2)]
        nc.gpsimd.memset(v_h[0], 1.0)
        nc.gpsimd.memset(v_h[1], 1.0)

        # ---- phase A: LN1 + QKV, per T-half (stats via ones-matmul) ----
        xnt_pool = tc.alloc_tile_pool(name="xnt", bufs=1)

        def emit_phaseA(half):
            hof = half * TH2
            xnt = xnt_pool.tile([128, DB, TH2], BF16, tag="xnt", name="xnt")
            for db in range(DB):
                dma(out=xnt[:, db, :],
                    in_=xT[128 * db:128 * db + 128, hof:hof + TH2])
            for tch in range(NTQ // 2):
                ts = slice(512 * tch, 512 * tch + 512)
                # pair: [:,0:512] = sum group, [:,512:1024] = sumsq group
                ps = pst("ps_stat")
                for db in range(DB):
                    sq = work2.tile([128, 512], BF16, tag="sq")
                    sq_eng = nc.vector if db < 2 else nc.gpsimd
                    sq_eng.tensor_mul(sq, xnt[:, db, ts], xnt[:, db, ts])
                    nc.tensor.matmul(ps[:, 0:512], ones128, xnt[:, db, ts],
                                     start=(db == 0), stop=(db == DB - 1))
                    nc.tensor.matmul(ps[:, 512:1024], ones128, sq,
                                     start=(db == 0), stop=(db == DB - 1))
                mu = work2.tile([128, 512], F32, tag="mu", bufs=1)
                nc.vector.tensor_scalar_mul(mu, ps[:, 0:512], 1.0 / D)
                mu2 = work2.tile([128, 512], F32, tag="mu2", bufs=1)
                nc.vector.tensor_mul(mu2, mu, mu)
                var = work2.tile([128, 512], F32, tag="var", bufs=1)
                nc.vector.scalar_tensor_tensor(var, ps[:, 512:1024], 1.0 / D,
                                               mu2, OP.mult, OP.subtract)
                nc.scalar.activation(var, var, AF.Sqrt, bias=eps128, scale=1.0)
                rstd = work2.tile([128, 512], F32, tag="rstd", bufs=1)
                nc.vector.reciprocal_approx_fast(rstd, var)
                nc.vector.tensor_mul(mu, mu, rstd)  # mu := mu*rstd
                for db in range(DB):
                    eng = nc.vector if db < 5 else nc.gpsimd
                    eng.tensor_mul(xnt[:, db, ts], xnt[:, db, ts], rstd)
                    eng.tensor_sub(xnt[:, db, ts], xnt[:, db, ts], mu)
                yield

            # q, k, v projections for this half
            for tch in range(NTQ // 2):
                ts = slice(512 * tch, 512 * tch + 512)
                # pair: [:,0:512] = q group, [:,512:1024] = k group
                ps = pst("ps_qk")
                for db in range(DB):
                    nc.tensor.matmul(ps[:, 0:512], wq_sb[:, db, :],
                                     xnt[:, db, ts],
                                     start=(db == 0), stop=(db == DB - 1))
                    nc.tensor.matmul(ps[:, 512:1024], wk_sb[:, db, :],
                                     xnt[:, db, ts],
                                     start=(db == 0), stop=(db == DB - 1))
                nc.scalar.activation(q_h[half][:, ts], ps[:, 0:512],
                                     AF.Identity, bias=bq_sb, scale=1.0)
                nc.scalar.activation(k_h[half][:, ts], ps[:, 512:1024],
                                     AF.Identity, bias=bk_sb, scale=1.0)
                yield
            for kt in range(NKH):
                tts = slice(128 * kt, 128 * kt + 128)
                psv = pst("psv")
                for db in range(DB):
                    nc.tensor.matmul(psv[:, 0:128], xnt[:, db, tts],
                                     wv_sb[:, db, :],
                                     start=(db == 0), stop=False)
                nc.tensor.matmul(psv[:, 0:128], ones1, bv_sb,
                                 start=False, stop=True)
                vv = v_h[half][:, kt * 2 * VW:(kt + 1) * 2 * VW].rearrange(
                    "p (h j) -> p h j", h=2)
                nc.scalar.copy(
                    vv[:, :, 0:HD],
                    psv[:, 0:128].rearrange("p (h j) -> p h j", h=2))
                if kt % 4 == 3:
                    yield

        # ---- FFN weights: pool allocated after xnt releases (SBUF room) ----
        ffnw_state = {}

        def emit_ffn_weights():
            ffnw = tc.alloc_tile_pool(name="ffnw", bufs=1)
            egt_sb, ept_sb, eot_sb = [], [], []
            for e in range(2):
                g = ffnw.tile([128, DB, NFB, 128], BF16, tag=f"egt{e}")
                dma(out=g, in_=egt[e]); egt_sb.append(g)
                p = ffnw.tile([128, DB, NFB, 128], BF16, tag=f"ept{e}")
                dma(out=p, in_=ept[e]); ept_sb.append(p)
                o = ffnw.tile([128, NFB, D], BF16, tag=f"eot{e}")
                dma(out=o, in_=eot[e]); eot_sb.append(o)
            ffnw_state.update(pool=ffnw, egt=egt_sb, ept=ept_sb, eot=eot_sb)

        yb, yr, pb, roA, roB = [], [], [], [], []
        for i in range(NCH):
            yb.append(dram.tile([128, DB, TC], BF16, tag=f"yb{i}", name=f"yb{i}"))
            yr.append(dram.tile([128, DB, TC], BF16, tag=f"yr{i}",
                                name=f"yr{i}", addr_space="Shared"))
            pb.append(dram.tile([DB, 128, TC], BF16, tag=f"pb{i}",
                                name=f"pb{i}"))
            roA.append(dram.tile([64, TC], BF16, tag=f"roA{i}", name=f"roA{i}"))
            roB.append(dram.tile([64, TC], BF16, tag=f"roB{i}", name=f"roB{i}"))

        rg = [list(range(NCORES))]

        def emit_att(ci):
            # ---- attention for chunk ci (two 512-token q-chunks) ----
            with nc.named_scope(f"att{ci}"):
                for tq in (2 * ci, 2 * ci + 1):
                    yT_sb = chunk2.tile([128, DB, 512], BF16, tag="yT",
                                        bufs=1, name="yT")
                    tqs0 = 512 * tq
                    hq, tql = tq // 4, tq % 4
                    pvt = pst("pvp")
                    pvp = pvt[0:VW, :]
                    pv = [pvp[:, 0:512], pvp[:, 512:1024]]
                    nkts = 4 * tq + 4

                    def emit_pv(kt, off, p_sb):
                        hk, ktl = kt // NKH, kt % NKH
                        for h in range(2):
                            vs = v_h[hk][:, ktl * 2 * VW + VW * h:
                                         ktl * 2 * VW + VW * h + VW]
                            nc.tensor.matmul(
                                pvp[:, 512 * h + off:512 * h + 512], vs,
                                p_sb[:, 512 * h + off:512 * h + 512],
                                start=(kt == 0),
                                stop=(kt == nkts - 1),
                                skip_group_check=True)

                    # kt loop software-pipelined: PV runs two kts behind the
                    # score/exp front so the in-order PE stream never waits
                    # on the ACT/DVE softmax of the current tile
                    pipe = []
                    for kt in range(nkts):
                        hk, ktl = kt // NKH, kt % NKH
                        kts = slice(128 * ktl, 128 * ktl + 128)
                        j = kt - 4 * tq
                        off = 128 * j if j > 0 else 0
                        ps_s = pst("ps_s")
                        for h in range(2):
                            nc.tensor.matmul(
                                ps_s[:, 512 * h + off:512 * h + 512],
                                k_h[hk][64 * h:64 * h + 64, kts],
                                q_h[hq][64 * h:64 * h + 64,
                                        512 * tql + off:512 * tql + 512],
                                start=True, stop=True,
                                tile_position=(64 * h, 0))
                        p_sb = work4.tile([128, 1024], BF16, tag="p",
                                          bufs=3, name="p_sb")
                        if off:
                            s3 = ps_s.rearrange("p (h t) -> p h t",
                                                h=2)[:, :, off:512]
                            p3 = p_sb.rearrange("p (h t) -> p h t",
                                                h=2)[:, :, off:512]
                            nc.scalar.activation(p3, s3, AF.Exp)
                        else:
                            nc.scalar.activation(p_sb, ps_s, AF.Exp)
                        if j >= 0:
                            for h in range(2):
                                nc.vector.tensor_mul(
                                    p_sb[:, 512 * h + off:512 * h + 512],
                                    p_sb[:, 512 * h + off:512 * h + 512],
                                    mask_sb[:, j, off:512])
                        pipe.append((kt, off, p_sb))
                        if kt >= 2:
                            emit_pv(*pipe[kt - 2])
                        yield
                    emit_pv(*pipe[nkts - 2])
                    emit_pv(*pipe[nkts - 1])
                    # softmax denominators -> normalized attn rows
                    lrow = work2.tile([2, 512], F32, tag="lrow", bufs=1,
                                      name="lrow")
                    for h in range(2):
                        ltmp = work2.tile([65, 512], F32, tag="ltmp", bufs=1,
                                          name="ltmp")
                        nc.scalar.copy(ltmp[64:65, :],
                                       pvp[HD:HD + 1, 512 * h:512 * h + 512])
                        dma(out=lrow[h:h + 1, :], in_=ltmp[64:65, :])
                    lrecf = work2.tile([2, 512], F32, tag="lrecf", bufs=1,
                                       name="lrecf")
                    nc.vector.reciprocal_approx_fast(lrecf, lrow)
                    lrec = work2.tile([2, 512], BF16, tag="lrec", bufs=1,
                                      name="lrec")
                    nc.vector.tensor_copy(lrec, lrecf)
                    at_sb = work2.tile([128, 512], BF16, tag="at", bufs=1,
                                       name="at")
                    atn1 = work2.tile([64, 512], BF16, tag="atn1", bufs=1,
                                      name="atn1")
                    ps_li = pst("ps_li")
                    for h in range(2):
                        nc.tensor.matmul(ps_li[0:64, 512 * h:512 * h + 512],
                                         sel2_sb[:, h, :],
                                         lrec, start=True, stop=True)
                    li2 = work4.tile([64, 1024], BF16, tag="li", bufs=2,
                                     name="li")
                    nc.scalar.copy(li2, ps_li[0:64, :])
                    nc.vector.tensor_mul(at_sb[0:64, :], pv[0][0:HD, :],
                                         li2[:, 0:512])
                    nc.vector.tensor_mul(atn1, pv[1][0:HD, :],
                                         li2[:, 512:1024])
                    dma(out=at_sb[64:128, :], in_=atn1)
                    # out-projection + x/8 -> y^T slices (AR then yields x2)
                    for dp in range(DB // 2):
                        ps_y = pst("ps_y")
                        for i2 in range(2):
                            db = 2 * dp + i2
                            nc.tensor.matmul(ps_y[:, 512 * i2:512 * i2 + 512],
                                             wo_sb[:, db, :], at_sb,
                                             start=True, stop=True)
                        for i2 in range(2):
                            db = 2 * dp + i2
                            xTc = work4.tile([128, 512], BF16, tag="xTc",
                                             bufs=2, name="xTc")
                            dma(out=xTc,
                                in_=xT[128 * db:128 * db + 128,
                                       tqs0:tqs0 + 512])
                            nc.vector.scalar_tensor_tensor(
                                yT_sb[:, db, :], xTc, 1.0 / NCORES,
                                ps_y[:, 512 * i2:512 * i2 + 512],
                                OP.mult, OP.add)
                        yield
                    dma(out=yb[ci][:, :, (tq % 2) * 512:(tq % 2) * 512 + 512],
                        in_=yT_sb)
                nc.gpsimd.collective_compute(
                    "AllReduce", OP.add, replica_groups=rg,
                    ins=[yb[ci][:]], outs=[yr[ci][:]])

        x2T_tiles = {}

        def emit_ffn_pro(ci):
            # ---- FFN prologue for chunk ci: x2 load + LN2 + router gates ----
            with nc.named_scope(f"ffnp{ci}"):
                x2T = chunk1.tile([128, DB, TC], BF16, tag="x2T", bufs=2,
                                  name="x2T")
                x2T_tiles[ci] = x2T
                dma(out=x2T, in_=yr[ci])
                yield
                gb = chunk1.tile([128, 2, TC], BF16, tag="gb", bufs=2,
                                 name="gb")
                x2T_tiles[f"gb{ci}"] = gb
                for th in range(2):
                    ths = slice(512 * th, 512 * th + 512)
                    # pair: [0:2,0:512] = sum rows, [0:2,512:1024] = dots rows
                    psAB = pst("psAB")
                    for db in range(DB):
                        nc.tensor.matmul(psAB[0:2, 0:512],
                                         ln2su_sb[:, db, :],
                                         x2T[:, db, ths],
                                         start=(db == 0), stop=(db == DB - 1))
                        nc.tensor.matmul(psAB[0:2, 512:1024],
                                         ln2do_sb[:, db, :],
                                         x2T[:, db, ths],
                                         start=(db == 0), stop=(db == DB - 1))
                    psC = pst("psC")
                    for db in range(DB):
                        sq = work2.tile([128, 512], BF16, tag="sq", name="sq")
                        nc.gpsimd.tensor_mul(sq, x2T[:, db, ths],
                                             x2T[:, db, ths])
                        nc.tensor.matmul(psC[0:2, 0:512],
                                         ln2su_sb[:, db, :], sq,
                                         start=(db == 0), stop=(db == DB - 1))
                    mu = work2.tile([2, 512], F32, tag="mu2r", bufs=1,
                                    name="mu2r")
                    nc.vector.tensor_scalar_mul(mu, psAB[0:2, 0:512], 1.0 / D)
                    mu2 = work2.tile([2, 512], F32, tag="mu2sq", bufs=1,
                                     name="mu2sq")
                    nc.vector.tensor_mul(mu2, mu, mu)
                    var = work2.tile([2, 512], F32, tag="var2", bufs=1,
                                     name="var2")
                    nc.vector.scalar_tensor_tensor(var, psC[0:2, 0:512],
                                                   1.0 / D, mu2,
                                                   OP.mult, OP.subtract)
                    nc.scalar.activation(var, var, AF.Sqrt, bias=eps2,
                                         scale=1.0)
                    rstd = work2.tile([2, 512], F32, tag="rstd2", bufs=1,
                                      name="rstd2")
                    nc.vector.reciprocal_approx_fast(rstd, var)
                    dotsS = work2.tile([2, 512], F32, tag="dotsS", bufs=1,
                                       name="dotsS")
                    nc.scalar.copy(dotsS, psAB[0:2, 512:1024])
                    zr = work2.tile([2, 512], F32, tag="zr", bufs=1, name="zr")
                    nc.vector.scalar_tensor_tensor(zr, psAB[0:2, 0:512],
                                                   scn_sb, dotsS,
                                                   OP.mult, OP.add)
                    zrb = work2.tile([2, 512], BF16, tag="zrb", bufs=1,
                                     name="zrb")
                    nc.vector.tensor_mul(zrb, zr, rstd)
                    ps_g = pst("ps_g")
                    for e in range(2):
                        nc.tensor.matmul(ps_g[:, 512 * e:512 * e + 512],
                                         sele_sb[:, e, :], zrb,
                                         start=True, stop=True)
                    for e in range(2):
                        nc.scalar.activation(gb[:, e, ths],
                                             ps_g[:, 512 * e:512 * e + 512],
                                             AF.Sigmoid,
                                             bias=cb_sb[:, e:e + 1], scale=1.0)
                    yield

        def emit_ffn_body(ci):
            # ---- FFN body for chunk ci: experts + out + ReduceScatter ----
            egt_sb = ffnw_state["egt"]
            ept_sb = ffnw_state["ept"]
            eot_sb = ffnw_state["eot"]
            x2T = x2T_tiles[ci]
            gb = x2T_tiles[f"gb{ci}"]
            with nc.named_scope(f"ffn{ci}"):
                # experts: h = relu(x2@egT) * (x2@epT) * gate
                hg_sb = [chunk1.tile([128, NFB, TC], BF16, tag=f"hg{e}",
                                     name=f"hg{e}") for e in range(2)]
                for e in range(2):
                    for fb in range(NFB):
                        ps_gm = pst("ps_gm")
                        for db in range(DB):
                            for th in range(2):
                                ths = slice(512 * th, 512 * th + 512)
                                nc.tensor.matmul(
                                    ps_gm[:, 512 * th:512 * th + 512],
                                    egt_sb[e][:, db, fb, :],
                                    x2T[:, db, ths],
                                    start=(db == 0), stop=(db == DB - 1))
                        r = work2.tile([128, 1024], BF16, tag="r", bufs=2,
                                       name="r")
                        nc.scalar.activation(r, ps_gm, AF.Relu)
                        ps_pm = pst("ps_pm")
                        for db in range(DB):
                            for th in range(2):
                                ths = slice(512 * th, 512 * th + 512)
                                nc.tensor.matmul(
                                    ps_pm[:, 512 * th:512 * th + 512],
                                    ept_sb[e][:, db, fb, :],
                                    x2T[:, db, ths],
                                    start=(db == 0), stop=(db == DB - 1))
                        hh = work2.tile([128, 1024], BF16, tag="hh", bufs=2,
                                        name="hh")
                        nc.vector.tensor_mul(hh, r, ps_pm)
                        nc.vector.tensor_mul(hg_sb[e][:, fb, :], hh,
                                             gb[:, e, :])
                        yield

                # out-experts, transposed: po^T[d, t] = x2^T/8 + sum_e eo_e h_e
                for db in range(DB):
                    ps_E = pst("ps_E")
                    for e in range(2):
                        for fb in range(NFB):
                            for th in range(2):
                                ths = slice(512 * th, 512 * th + 512)
                                nc.tensor.matmul(
                                    ps_E[:, 512 * th:512 * th + 512],
                                    eot_sb[e][:, fb,
                                              128 * db:128 * db + 128],
                                    hg_sb[e][:, fb, ths],
                                    start=(e == 0 and fb == 0),
                                    stop=(e == 1 and fb == NFB - 1))
                    po = work2.tile([128, TC], BF16, tag="po", bufs=2,
                                    name="po")
                    nc.vector.scalar_tensor_tensor(
                        po, x2T[:, db, :], 1.0 / NCORES, ps_E,
                        OP.mult, OP.add)
                    dma(out=pb[ci][db], in_=po)
                    if db == DB // 2 - 1:
                        nc.gpsimd.collective_compute(
                            "ReduceScatter", OP.add, replica_groups=rg,
                            ins=[pb[ci][0:DB // 2]], outs=[roA[ci][:]])
                        dma(out=out_rows[ci][0], in_=roA[ci][:])
                    yield
                nc.gpsimd.collective_compute(
                    "ReduceScatter", OP.add, replica_groups=rg,
                    ins=[pb[ci][DB // 2:DB]], outs=[roB[ci][:]])
                dma(out=out_rows[ci][1], in_=roB[ci][:])

        # Emission order = per-engine execution order (static streams), so
        # emit work in the order its data becomes ready.
        def drain(g):
            for _ in g:
                pass

        def chain(*gens):
            for g in gens:
                yield from g

        def interleave(*gens):
            live = list(gens)
            while live:
                for g in list(live):
                    try:
                        next(g)
                    except StopIteration:
                        live.remove(g)

        drain(emit_phaseA(0))
        drain(emit_att(0))
        interleave(emit_att(1), emit_phaseA(1))
        xnt_pool.release()
        emit_ffn_weights()
        interleave(emit_att(2), emit_ffn_pro(0))
        interleave(emit_att(3), emit_ffn_body(0))
        drain(emit_ffn_pro(1))
        interleave(emit_ffn_body(1), emit_ffn_pro(2))
        interleave(emit_ffn_body(2), emit_ffn_pro(3))
        drain(emit_ffn_body(3))

        for p in (ffnw_state["pool"], dram, psB,
                  chunk2, chunk1, work4, work2, const):
            p.release()

    nc.compile()
    return nc


def _prep_inputs(inputs):
    """Build the 8 per-core input maps (host-side sharding / layout prep)."""
    f32 = np.float32

    def np32(a):
        return np.asarray(a, dtype=f32)

    x = np32(inputs["x"])[0]                      # [T, D]
    ln1_w, ln1_b = np32(inputs["ln1_w"]), np32(inputs["ln1_b"])
    ln2_w, ln2_b = np32(inputs["ln2_w"]), np32(inputs["ln2_b"])
    Wq, Wk, Wv, Wo = (np32(inputs[k]) for k in ("Wq", "Wk", "Wv", "Wo"))
    router_w, router_b = np32(inputs["router_w"]), np32(inputs["router_b"])
    eg, ep, eo = np32(inputs["eg"]), np32(inputs["ep"]), np32(inputs["eo"])

    xT = np.ascontiguousarray(x.T).astype(NPBF16)          # [D, T]

    scale_q = 1.0 / np.sqrt(HD)
    rw_eff = router_w * ln2_w[None, :]                     # [2, D]
    S = rw_eff.sum(axis=1)                                 # [2]
    c_e = router_b + router_w @ ln2_b                      # [2]
    scn = (-(S / D)).reshape(2, 1).astype(f32)
    cbias = np.broadcast_to(c_e[None, :], (128, 2)).astype(f32).copy()

    ln2su = np.ones((128, DB, 2), f32)
    ln2do = np.zeros((128, DB, 2), f32)
    rw_r = rw_eff.reshape(2, DB, 128)                      # [e, db, p]
    ln2do[:, :, 0] = rw_r[0].T
    ln2do[:, :, 1] = rw_r[1].T

    masks = np.zeros((128, 4, 512), f32)
    p_i = np.arange(128)[:, None]
    t_i = np.arange(512)[None, :]
    for j in range(4):
        masks[:, j, :] = (t_i >= 128 * j + p_i)

    sel2b = np.zeros((2, 2, 64), f32)                      # [j, h, m] = (j==h)
    sel2b[0, 0, :] = 1.0
    sel2b[1, 1, :] = 1.0
    sele = np.zeros((2, 2, 128), f32)                      # [j, e, m] = (j==e)
    sele[0, 0, :] = 1.0
    sele[1, 1, :] = 1.0

    def stat_pack(Wsh):  # [128(m), D] -> [128(kp), DB, 128(m)] lhsT layout
        return np.ascontiguousarray(
            Wsh.T.reshape(DB, 128, 128).transpose(1, 0, 2))

    in_maps = []
    for c in range(NCORES):
        hs = slice(128 * c, 128 * c + 128)
        Wq_sh = (Wq * ln1_w[None, :])[hs] * scale_q        # [128, D]
        Wk_sh = (Wk * ln1_w[None, :])[hs]
        Wv_sh = (Wv * ln1_w[None, :])[hs]
        bq = (Wq[hs] @ ln1_b) * scale_q
        bk = Wk[hs] @ ln1_b
        bv = Wv[hs] @ ln1_b
        Wo_sh = Wo[:, hs]                                  # [D, 128]
        wo_pack = np.ascontiguousarray(
            Wo_sh.reshape(DB, 128, 128).transpose(2, 0, 1))  # [i, db, m]

        fs = slice(FFS * c, FFS * c + FFS)
        egt = np.stack([
            np.ascontiguousarray(
                eg[e][fs].T.reshape(DB, 128, NFB, 128).transpose(1, 0, 2, 3))
            for e in range(2)])
        ept = np.stack([
            np.ascontiguousarray(
                ep[e][fs].T.reshape(DB, 128, NFB, 128).transpose(1, 0, 2, 3))
            for e in range(2)])
        eot = np.stack([
            np.ascontiguousarray(
                eo[e][:, fs].T.reshape(NFB, 128, D).transpose(1, 0, 2))
            for e in range(2)])

        in_maps.append({
            "xT": xT,
            "wq": stat_pack(Wq_sh).astype(NPBF16),
            "wk": stat_pack(Wk_sh).astype(NPBF16),
            "wv": stat_pack(Wv_sh).astype(NPBF16),
            "bq": bq.reshape(128, 1).astype(f32),
            "bk": bk.reshape(128, 1).astype(f32),
            "bv": bv.reshape(1, 128).astype(NPBF16),
            "wo": wo_pack.astype(NPBF16),
            "ln2su": ln2su.astype(NPBF16),
            "ln2do": ln2do.astype(NPBF16),
            "scn": scn, "cbias": cbias,
            "masks": masks.astype(NPBF16),
            "sel2b": sel2b.astype(NPBF16),
            "sele": sele.astype(NPBF16),
            "egt": egt.astype(NPBF16),
            "ept": ept.astype(NPBF16),
            "eot": eot.astype(NPBF16),
        })
    return in_maps


def _get_compiled():
    global _COMPILED
    if _COMPILED is None:
        _COMPILED = _build_nc()
    return _COMPILED


def _unshard(results):
    out = np.zeros((NCH, TC, D), np.float32)
    for c in range(NCORES):
        r = np.asarray(results[c]["out_rows"], dtype=np.float32)
        # r[ci, half, i, t] -> out[ci, t, 512*half + 64*c + i]
        for i in range(NCH):
            out[i, :, 64 * c:64 * c + 64] = r[i, 0].T
            out[i, :, 512 + 64 * c:512 + 64 * c + 64] = r[i, 1].T
    return out.reshape(B, T, D)


def kernel(**inputs):
    nc = _get_compiled()
    in_maps = _prep_inputs(inputs)
    res = run_bass_kernel_spmd(nc, in_maps, list(range(NCORES)))
    return _unshard(res.results)


# revision 23
# speedup vs baseline: 1.0523x; 1.0523x over previous
"""Trainium2 Bass kernel for nn_MitoticTransformerBlock (full causal attention +
soft-gated 2-expert FFN), sharded over 8 NeuronCores.

Sharding: attention by heads (2 heads/core), experts tensor-parallel over the ff
dim (512/core/expert).  Each core folds x/8 into its out-proj partial, so the
chunked AllReduce directly yields x2 = x + attn on every core; expert partials
(+x2/8) are ReduceScattered in bf16 (transposed layout, two halves per chunk)
so each core ends up with a 64-row d-slice of every token's final output.

v4: one 4-slot [128,1024] PSUM pool (PV accumulator included), paired matmul
groups (two 512-wide groups per PSUM tile) halving ACT/DVE op counts,
readiness-ordered emission with FFN prologue/body split and double-buffered
x2T, split ReduceScatter to shrink the exposed tail.
"""

import sys

sys.path.insert(0, "/opt/trn_rl_repo")

import numpy as np
import ml_dtypes

import concourse.bass as bass
import concourse.tile as tile
import concourse.mybir as mybir
from concourse import bacc
from concourse.bass_utils import run_bass_kernel_spmd

F32 = mybir.dt.float32
BF16 = mybir.dt.bfloat16
AF = mybir.ActivationFunctionType
OP = mybir.AluOpType
NPBF16 = ml_dtypes.bfloat16

NCORES = 8
B, T, D, H, FF = 1, 4096, 1024, 16, 4096
HD = D // H          # 64
DB = D // 128        # 8 d-blocks
NTQ = T // 512       # 8 attention q-chunks of 512
NKT = T // 128       # 32 key tiles
TC = 1024            # AllReduce/FFN chunk (tokens)
NCH = T // TC        # 4 chunks
FFS = FF // NCORES   # 512 ff slice per core per expert
NFB = FFS // 128     # 4 ff blocks
LN_EPS = 1e-5
VW = HD + 1          # 65: v columns + ones column
TH2 = T // 2
NKH = NKT // 2

_COMPILED = None


def _build_nc():
    nc = bacc.Bacc("TRN2", target_bir_lowering=False, debug=False,
                   num_devices=NCORES)

    def din(name, shape, dt):
        return nc.dram_tensor(name, shape, dt, kind="ExternalInput").ap()

    xT = din("xT", [D, T], BF16)
    wq = din("wq", [128, DB, 128], BF16)
    wk = din("wk", [128, DB, 128], BF16)
    wv = din("wv", [128, DB, 128], BF16)
    bq = din("bq", [128, 1], F32)
    bk = din("bk", [128, 1], F32)
    bv = din("bv", [1, 128], BF16)
    wo = din("wo", [128, DB, 128], BF16)
    ln2su = din("ln2su", [128, DB, 2], BF16)
    ln2do = din("ln2do", [128, DB, 2], BF16)
    scn = din("scn", [2, 1], F32)
    cbias = din("cbias", [128, 2], F32)
    masks = din("masks", [128, 4, 512], BF16)
    sel2b = din("sel2b", [2, 2, 64], BF16)
    sele = din("sele", [2, 2, 128], BF16)
    egt = din("egt", [2, 128, DB, NFB, 128], BF16)
    ept = din("ept", [2, 128, DB, NFB, 128], BF16)
    eot = din("eot", [2, 128, NFB, D], BF16)

    out_rows = nc.dram_tensor("out_rows", [NCH, 2, 64, TC], BF16,
                              kind="ExternalOutput").ap()

    with tile.TileContext(nc) as tc:
        const = tc.alloc_tile_pool(name="const", bufs=1)
        work2 = tc.alloc_tile_pool(name="work2", bufs=2)
        work4 = tc.alloc_tile_pool(name="work4", bufs=4)
        chunk1 = tc.alloc_tile_pool(name="chunk1", bufs=1)
        chunk2 = tc.alloc_tile_pool(name="chunk2", bufs=2)
        psB = tc.alloc_tile_pool(name="psB", bufs=4, space="PSUM")
        dram = tc.alloc_tile_pool(name="dram", bufs=1, space="DRAM")

        dma = nc.sync.dma_start

        def pst(name):
            return psB.tile([128, 1024], F32, tag="b", name=name)

        # ---- setup: constants into SBUF ----
        wq_sb = const.tile([128, DB, 128], BF16); dma(out=wq_sb, in_=wq)
        wk_sb = const.tile([128, DB, 128], BF16); dma(out=wk_sb, in_=wk)
        wv_sb = const.tile([128, DB, 128], BF16); dma(out=wv_sb, in_=wv)
        bq_sb = const.tile([128, 1], F32); dma(out=bq_sb, in_=bq)
        bk_sb = const.tile([128, 1], F32); dma(out=bk_sb, in_=bk)
        bv_sb = const.tile([1, 128], BF16); dma(out=bv_sb, in_=bv)
        wo_sb = const.tile([128, DB, 128], BF16); dma(out=wo_sb, in_=wo)
        ln2su_sb = const.tile([128, DB, 2], BF16); dma(out=ln2su_sb, in_=ln2su)
        ln2do_sb = const.tile([128, DB, 2], BF16); dma(out=ln2do_sb, in_=ln2do)
        scn_sb = const.tile([2, 1], F32); dma(out=scn_sb, in_=scn)
        cb_sb = const.tile([128, 2], F32); dma(out=cb_sb, in_=cbias)
        mask_sb = const.tile([128, 4, 512], BF16); dma(out=mask_sb, in_=masks)
        sel2_sb = const.tile([2, 2, 64], BF16); dma(out=sel2_sb, in_=sel2b)
        sele_sb = const.tile([2, 2, 128], BF16); dma(out=sele_sb, in_=sele)
        ones128 = const.tile([128, 128], BF16)
        nc.gpsimd.memset(ones128, 1.0)
        ones1 = const.tile([1, 128], BF16)
        nc.gpsimd.memset(ones1, 1.0)
        eps2 = const.tile([2, 1], F32)
        nc.gpsimd.memset(eps2, LN_EPS)
        eps128 = const.tile([128, 1], F32)
        nc.gpsimd.memset(eps128, LN_EPS)

        q_h = [const.tile([128, TH2], BF16, name=f"q{i}") for i in range(2)]
        k_h = [const.tile([128, TH2], BF16, name=f"k{i}") for i in range(2)]
        v_h = [const.tile([128, NKH * 2 * VW], BF16, name=f"v{i}")
               for i in range(2)]
        nc.gpsimd.memset(v_h[0], 1.0)
        nc.gpsimd.memset(v_h[1], 1.0)

        # ---- phase A: LN1 + QKV, per T-half (stats via ones-matmul) ----
        xnt_pool = tc.alloc_tile_pool(name="xnt", bufs=1)

        def emit_phaseA(half):
            hof = half * TH2
            xnt = xnt_pool.tile([128, DB, TH2], BF16, tag="xnt", name="xnt")
            for db in range(DB):
                dma(out=xnt[:, db, :],
                    in_=xT[128 * db:128 * db + 128, hof:hof + TH2])
            for tch in range(NTQ // 2):
                ts = slice(512 * tch, 512 * tch + 512)
                # pair: [:,0:512] = sum group, [:,512:1024] = sumsq group
                ps = pst("ps_stat")
                for db in range(DB):
                    sq = work2.tile([128, 512], BF16, tag="sq")
                    nc.vector.tensor_mul(sq, xnt[:, db, ts], xnt[:, db, ts])
                    nc.tensor.matmul(ps[:, 0:512], ones128, xnt[:, db, ts],
                                     start=(db == 0), stop=(db == DB - 1))
                    nc.tensor.matmul(ps[:, 512:1024], ones128, sq,
                                     start=(db == 0), stop=(db == DB - 1))
                mu = work2.tile([128, 512], F32, tag="mu", bufs=1)
                nc.vector.tensor_scalar_mul(mu, ps[:, 0:512], 1.0 / D)
                mu2 = work2.tile([128, 512], F32, tag="mu2", bufs=1)
                nc.vector.tensor_mul(mu2, mu, mu)
                var = work2.tile([128, 512], F32, tag="var", bufs=1)
                nc.vector.scalar_tensor_tensor(var, ps[:, 512:1024], 1.0 / D,
                                               mu2, OP.mult, OP.subtract)
                nc.scalar.activation(var, var, AF.Sqrt, bias=eps128, scale=1.0)
                rstd = work2.tile([128, 512], F32, tag="rstd", bufs=1)
                nc.vector.reciprocal_approx_fast(rstd, var)
                nc.vector.tensor_mul(mu, mu, rstd)  # mu := mu*rstd
                for db in range(DB):
                    nc.vector.tensor_mul(xnt[:, db, ts], xnt[:, db, ts], rstd)
                    nc.vector.tensor_sub(xnt[:, db, ts], xnt[:, db, ts], mu)
                yield

            # q, k, v projections for this half
            for tch in range(NTQ // 2):
                ts = slice(512 * tch, 512 * tch + 512)
                # pair: [:,0:512] = q group, [:,512:1024] = k group
                ps = pst("ps_qk")
                for db in range(DB):
                    nc.tensor.matmul(ps[:, 0:512], wq_sb[:, db, :],
                                     xnt[:, db, ts],
                                     start=(db == 0), stop=(db == DB - 1))
                    nc.tensor.matmul(ps[:, 512:1024], wk_sb[:, db, :],
                                     xnt[:, db, ts],
                                     start=(db == 0), stop=(db == DB - 1))
                nc.scalar.activation(q_h[half][:, ts], ps[:, 0:512],
                                     AF.Identity, bias=bq_sb, scale=1.0)
                nc.scalar.activation(k_h[half][:, ts], ps[:, 512:1024],
                                     AF.Identity, bias=bk_sb, scale=1.0)
                yield
            for kt in range(NKH):
                tts = slice(128 * kt, 128 * kt + 128)
                psv = pst("psv")
                for db in range(DB):
                    nc.tensor.matmul(psv[:, 0:128], xnt[:, db, tts],
                                     wv_sb[:, db, :],
                                     start=(db == 0), stop=False)
                nc.tensor.matmul(psv[:, 0:128], ones1, bv_sb,
                                 start=False, stop=True)
                vv = v_h[half][:, kt * 2 * VW:(kt + 1) * 2 * VW].rearrange(
                    "p (h j) -> p h j", h=2)
                nc.scalar.copy(
                    vv[:, :, 0:HD],
                    psv[:, 0:128].rearrange("p (h j) -> p h j", h=2))
                if kt % 4 == 3:
                    yield

        # ---- FFN weights: pool allocated after xnt releases (SBUF room) ----
        ffnw_state = {}

        def emit_ffn_weights():
            ffnw = tc.alloc_tile_pool(name="ffnw", bufs=1)
            egt_sb, ept_sb, eot_sb = [], [], []
            for e in range(2):
                g = ffnw.tile([128, DB, NFB, 128], BF16, tag=f"egt{e}")
                dma(out=g, in_=egt[e]); egt_sb.append(g)
                p = ffnw.tile([128, DB, NFB, 128], BF16, tag=f"ept{e}")
                dma(out=p, in_=ept[e]); ept_sb.append(p)
                o = ffnw.tile([128, NFB, D], BF16, tag=f"eot{e}")
                dma(out=o, in_=eot[e]); eot_sb.append(o)
            ffnw_state.update(pool=ffnw, egt=egt_sb, ept=ept_sb, eot=eot_sb)

        yb, yr, pb, roA, roB = [], [], [], [], []
        for i in range(NCH):
            yb.append(dram.tile([128, DB, TC], BF16, tag=f"yb{i}", name=f"yb{i}"))
            yr.append(dram.tile([128, DB, TC], BF16, tag=f"yr{i}",
                                name=f"yr{i}", addr_space="Shared"))
            pb.append(dram.tile([DB, 128, TC], BF16, tag=f"pb{i}",
                                name=f"pb{i}"))
            roA.append(dram.tile([64, TC], BF16, tag=f"roA{i}", name=f"roA{i}"))
            roB.append(dram.tile([64, TC], BF16, tag=f"roB{i}", name=f"roB{i}"))

        rg = [list(range(NCORES))]

        def emit_att(ci):
            # ---- attention for chunk ci (two 512-token q-chunks) ----
            with nc.named_scope(f"att{ci}"):
                for tq in (2 * ci, 2 * ci + 1):
                    yT_sb = chunk2.tile([128, DB, 512], BF16, tag="yT",
                                        bufs=1, name="yT")
                    tqs0 = 512 * tq
                    hq, tql = tq // 4, tq % 4
                    pvt = pst("pvp")
                    pvp = pvt[0:VW, :]
                    pv = [pvp[:, 0:512], pvp[:, 512:1024]]
                    nkts = 4 * tq + 4

                    def emit_pv(kt, off, p_sb):
                        hk, ktl = kt // NKH, kt % NKH
                        for h in range(2):
                            vs = v_h[hk][:, ktl * 2 * VW + VW * h:
                                         ktl * 2 * VW + VW * h + VW]
                            nc.tensor.matmul(
                                pvp[:, 512 * h + off:512 * h + 512], vs,
                                p_sb[:, 512 * h + off:512 * h + 512],
                                start=(kt == 0),
                                stop=(kt == nkts - 1),
                                skip_group_check=True)

                    # kt loop software-pipelined: PV runs two kts behind the
                    # score/exp front so the in-order PE stream never waits
                    # on the ACT/DVE softmax of the current tile
                    pipe = []
                    for kt in range(nkts):
                        hk, ktl = kt // NKH, kt % NKH
                        kts = slice(128 * ktl, 128 * ktl + 128)
                        j = kt - 4 * tq
                        off = 128 * j if j > 0 else 0
                        ps_s = pst("ps_s")
                        for h in range(2):
                            nc.tensor.matmul(
                                ps_s[:, 512 * h + off:512 * h + 512],
                                k_h[hk][64 * h:64 * h + 64, kts],
                                q_h[hq][64 * h:64 * h + 64,
                                        512 * tql + off:512 * tql + 512],
                                start=True, stop=True,
                                tile_position=(64 * h, 0))
                        p_sb = work4.tile([128, 1024], BF16, tag="p",
                                          bufs=3, name="p_sb")
                        if off:
                            s3 = ps_s.rearrange("p (h t) -> p h t",
                                                h=2)[:, :, off:512]
                            p3 = p_sb.rearrange("p (h t) -> p h t",
                                                h=2)[:, :, off:512]
                            nc.scalar.activation(p3, s3, AF.Exp)
                        else:
                            nc.scalar.activation(p_sb, ps_s, AF.Exp)
                        if j >= 0:
                            for h in range(2):
                                nc.vector.tensor_mul(
                                    p_sb[:, 512 * h + off:512 * h + 512],
                                    p_sb[:, 512 * h + off:512 * h + 512],
                                    mask_sb[:, j, off:512])
                        pipe.append((kt, off, p_sb))
                        if kt >= 2:
                            emit_pv(*pipe[kt - 2])
                        yield
                    emit_pv(*pipe[nkts - 2])
                    emit_pv(*pipe[nkts - 1])
                    # softmax denominators -> normalized attn rows
                    lrow = work2.tile([2, 512], F32, tag="lrow", bufs=1,
                                      name="lrow")
                    for h in range(2):
                        ltmp = work2.tile([65, 512], F32, tag="ltmp", bufs=1,
                                          name="ltmp")
                        nc.scalar.copy(ltmp[64:65, :],
                                       pvp[HD:HD + 1, 512 * h:512 * h + 512])
                        dma(out=lrow[h:h + 1, :], in_=ltmp[64:65, :])
                    lrecf = work2.tile([2, 512], F32, tag="lrecf", bufs=1,
                                       name="lrecf")
                    nc.vector.reciprocal_approx_fast(lrecf, lrow)
                    lrec = work2.tile([2, 512], BF16, tag="lrec", bufs=1,
                                      name="lrec")
                    nc.vector.tensor_copy(lrec, lrecf)
                    at_sb = work2.tile([128, 512], BF16, tag="at", bufs=1,
                                       name="at")
                    atn1 = work2.tile([64, 512], BF16, tag="atn1", bufs=1,
                                      name="atn1")
                    ps_li = pst("ps_li")
                    for h in range(2):
                        nc.tensor.matmul(ps_li[0:64, 512 * h:512 * h + 512],
                                         sel2_sb[:, h, :],
                                         lrec, start=True, stop=True)
                    li2 = work4.tile([64, 1024], BF16, tag="li", bufs=2,
                                     name="li")
                    nc.scalar.copy(li2, ps_li[0:64, :])
                    nc.vector.tensor_mul(at_sb[0:64, :], pv[0][0:HD, :],
                                         li2[:, 0:512])
                    nc.vector.tensor_mul(atn1, pv[1][0:HD, :],
                                         li2[:, 512:1024])
                    dma(out=at_sb[64:128, :], in_=atn1)
                    # out-projection + x/8 -> y^T slices (AR then yields x2)
                    for dp in range(DB // 2):
                        ps_y = pst("ps_y")
                        for i2 in range(2):
                            db = 2 * dp + i2
                            nc.tensor.matmul(ps_y[:, 512 * i2:512 * i2 + 512],
                                             wo_sb[:, db, :], at_sb,
                                             start=True, stop=True)
                        for i2 in range(2):
                            db = 2 * dp + i2
                            xTc = work4.tile([128, 512], BF16, tag="xTc",
                                             bufs=2, name="xTc")
                            dma(out=xTc,
                                in_=xT[128 * db:128 * db + 128,
                                       tqs0:tqs0 + 512])
                            nc.vector.scalar_tensor_tensor(
                                yT_sb[:, db, :], xTc, 1.0 / NCORES,
                                ps_y[:, 512 * i2:512 * i2 + 512],
                                OP.mult, OP.add)
                        yield
                    dma(out=yb[ci][:, :, (tq % 2) * 512:(tq % 2) * 512 + 512],
                        in_=yT_sb)
                nc.gpsimd.collective_compute(
                    "AllReduce", OP.add, replica_groups=rg,
                    ins=[yb[ci][:]], outs=[yr[ci][:]])

        x2T_tiles = {}

        def emit_ffn_pro(ci):
            # ---- FFN prologue for chunk ci: x2 load + LN2 + router gates ----
            with nc.named_scope(f"ffnp{ci}"):
                x2T = chunk1.tile([128, DB, TC], BF16, tag="x2T", bufs=2,
                                  name="x2T")
                x2T_tiles[ci] = x2T
                dma(out=x2T, in_=yr[ci])
                yield
                gb = chunk1.tile([128, 2, TC], BF16, tag="gb", bufs=2,
                                 name="gb")
                x2T_tiles[f"gb{ci}"] = gb
                for th in range(2):
                    ths = slice(512 * th, 512 * th + 512)
                    # pair: [0:2,0:512] = sum rows, [0:2,512:1024] = dots rows
                    psAB = pst("psAB")
                    for db in range(DB):
                        nc.tensor.matmul(psAB[0:2, 0:512],
                                         ln2su_sb[:, db, :],
                                         x2T[:, db, ths],
                                         start=(db == 0), stop=(db == DB - 1))
                        nc.tensor.matmul(psAB[0:2, 512:1024],
                                         ln2do_sb[:, db, :],
                                         x2T[:, db, ths],
                                         start=(db == 0), stop=(db == DB - 1))
                    psC = pst("psC")
                    for db in range(DB):
                        sq = work2.tile([128, 512], BF16, tag="sq", name="sq")
                        nc.vector.tensor_mul(sq, x2T[:, db, ths],
                                             x2T[:, db, ths])
                        nc.tensor.matmul(psC[0:2, 0:512],
                                         ln2su_sb[:, db, :], sq,
                                         start=(db == 0), stop=(db == DB - 1))
                    mu = work2.tile([2, 512], F32, tag="mu2r", bufs=1,
                                    name="mu2r")
                    nc.vector.tensor_scalar_mul(mu, psAB[0:2, 0:512], 1.0 / D)
                    mu2 = work2.tile([2, 512], F32, tag="mu2sq", bufs=1,
                                     name="mu2sq")
                    nc.vector.tensor_mul(mu2, mu, mu)
                    var = work2.tile([2, 512], F32, tag="var2", bufs=1,
                                     name="var2")
                    nc.vector.scalar_tensor_tensor(var, psC[0:2, 0:512],
                                                   1.0 / D, mu2,
                                                   OP.mult, OP.subtract)
                    nc.scalar.activation(var, var, AF.Sqrt, bias=eps2,
                                         scale=1.0)
                    rstd = work2.tile([2, 512], F32, tag="rstd2", bufs=1,
                                      name="rstd2")
                    nc.vector.reciprocal_approx_fast(rstd, var)
                    dotsS = work2.tile([2, 512], F32, tag="dotsS", bufs=1,
                                       name="dotsS")
                    nc.scalar.copy(dotsS, psAB[0:2, 512:1024])
                    zr = work2.tile([2, 512], F32, tag="zr", bufs=1, name="zr")
                    nc.vector.scalar_tensor_tensor(zr, psAB[0:2, 0:512],
                                                   scn_sb, dotsS,
                                                   OP.mult, OP.add)
                    zrb = work2.tile([2, 512], BF16, tag="zrb", bufs=1,
                                     name="zrb")
                    nc.vector.tensor_mul(zrb, zr, rstd)
                    ps_g = pst("ps_g")
                    for e in range(2):
                        nc.tensor.matmul(ps_g[:, 512 * e:512 * e + 512],
                                         sele_sb[:, e, :], zrb,
                                         start=True, stop=True)
                    for e in range(2):
                        nc.scalar.activation(gb[:, e, ths],
                                             ps_g[:, 512 * e:512 * e + 512],
                                             AF.Sigmoid,
                                             bias=cb_sb[:, e:e + 1], scale=1.0)
                    yield

        def emit_ffn_body(ci):
            # ---- FFN body for chunk ci: experts + out + ReduceScatter ----
            egt_sb = ffnw_state["egt"]
            ept_sb = ffnw_state["ept"]
            eot_sb = ffnw_state["eot"]
            x2T = x2T_tiles[ci]
            gb = x2T_tiles[f"gb{ci}"]
            with nc.named_scope(f"ffn{ci}"):
                # experts: h = relu(x2@egT) * (x2@epT) * gate
                hg_sb = [chunk1.tile([128, NFB, TC], BF16, tag=f"hg{e}",
                                     name=f"hg{e}") for e in range(2)]
                for e in range(2):
                    for fb in range(NFB):
                        ps_gm = pst("ps_gm")
                        for db in range(DB):
                            for th in range(2):
                                ths = slice(512 * th, 512 * th + 512)
                                nc.tensor.matmul(
                                    ps_gm[:, 512 * th:512 * th + 512],
                                    egt_sb[e][:, db, fb, :],
                                    x2T[:, db, ths],
                                    start=(db == 0), stop=(db == DB - 1))
                        r = work2.tile([128, 1024], BF16, tag="r", bufs=2,
                                       name="r")
                        nc.scalar.activation(r, ps_gm, AF.Relu)
                        ps_pm = pst("ps_pm")
                        for db in range(DB):
                            for th in range(2):
                                ths = slice(512 * th, 512 * th + 512)
                                nc.tensor.matmul(
                                    ps_pm[:, 512 * th:512 * th + 512],
                                    ept_sb[e][:, db, fb, :],
                                    x2T[:, db, ths],
                                    start=(db == 0), stop=(db == DB - 1))
                        hh = work2.tile([128, 1024], BF16, tag="hh", bufs=2,
                                        name="hh")
                        nc.vector.tensor_mul(hh, r, ps_pm)
                        nc.vector.tensor_mul(hg_sb[e][:, fb, :], hh,
                                             gb[:, e, :])
                        yield

                # out-experts, transposed: po^T[d, t] = x2^T/8 + sum_e eo_e h_e
                for db in range(DB):
                    ps_E = pst("ps_E")
                    for e in range(2):
                        for fb in range(NFB):
                            for th in range(2):
                                ths = slice(512 * th, 512 * th + 512)
                                nc.tensor.matmul(
                                    ps_E[:, 512 * th:512 * th + 512],
                                    eot_sb[e][:, fb,
                                              128 * db:128 * db + 128],
                                    hg_sb[e][:, fb, ths],
                                    start=(e == 0 and fb == 0),
                                    stop=(e == 1 and fb == NFB - 1))
                    po = work2.tile([128, TC], BF16, tag="po", bufs=2,
                                    name="po")
                    nc.vector.scalar_tensor_tensor(
                        po, x2T[:, db, :], 1.0 / NCORES, ps_E,
                        OP.mult, OP.add)
                    dma(out=pb[ci][db], in_=po)
                    if db == DB // 2 - 1:
                        nc.gpsimd.collective_compute(
                            "ReduceScatter", OP.add, replica_groups=rg,
                            ins=[pb[ci][0:DB // 2]], outs=[roA[ci][:]])
                        dma(out=out_rows[ci][0], in_=roA[ci][:])
                    yield
                nc.gpsimd.collective_compute(
                    "ReduceScatter", OP.add, replica_groups=rg,
                    ins=[pb[ci][DB // 2:DB]], outs=[roB[ci][:]])
                dma(out=out_rows[ci][1], in_=roB[ci][:])

        # Emission order = per-engine execution order (static streams), so
        # emit work in the order its data becomes ready.
        def drain(g):
            for _ in g:
                pass

        def chain(*gens):
            for g in gens:
                yield from g

        def interleave(*gens):
            live = list(gens)
            while live:
                for g in list(live):
                    try:
                        next(g)
                    except StopIteration:
                        live.remove(g)

        drain(emit_phaseA(0))
        drain(emit_att(0))
        interleave(emit_att(1), emit_phaseA(1))
        xnt_pool.release()
        emit_ffn_weights()
        interleave(emit_att(2), emit_ffn_pro(0))
        interleave(emit_att(3), emit_ffn_body(0))
        drain(emit_ffn_pro(1))
        interleave(emit_ffn_body(1), emit_ffn_pro(2))
        interleave(emit_ffn_body(2), emit_ffn_pro(3))
        drain(emit_ffn_body(3))

        for p in (ffnw_state["pool"], dram, psB,
                  chunk2, chunk1, work4, work2, const):
            p.release()

    nc.compile()
    return nc


def _prep_inputs(inputs):
    """Build the 8 per-core input maps (host-side sharding / layout prep)."""
    f32 = np.float32

    def np32(a):
        return np.asarray(a, dtype=f32)

    x = np32(inputs["x"])[0]                      # [T, D]
    ln1_w, ln1_b = np32(inputs["ln1_w"]), np32(inputs["ln1_b"])
    ln2_w, ln2_b = np32(inputs["ln2_w"]), np32(inputs["ln2_b"])
    Wq, Wk, Wv, Wo = (np32(inputs[k]) for k in ("Wq", "Wk", "Wv", "Wo"))
    router_w, router_b = np32(inputs["router_w"]), np32(inputs["router_b"])
    eg, ep, eo = np32(inputs["eg"]), np32(inputs["ep"]), np32(inputs["eo"])

    xT = np.ascontiguousarray(x.T).astype(NPBF16)          # [D, T]

    scale_q = 1.0 / np.sqrt(HD)
    rw_eff = router_w * ln2_w[None, :]                     # [2, D]
    S = rw_eff.sum(axis=1)                                 # [2]
    c_e = router_b + router_w @ ln2_b                      # [2]
    scn = (-(S / D)).reshape(2, 1).astype(f32)
    cbias = np.broadcast_to(c_e[None, :], (128, 2)).astype(f32).copy()

    ln2su = np.ones((128, DB, 2), f32)
    ln2do = np.zeros((128, DB, 2), f32)
    rw_r = rw_eff.reshape(2, DB, 128)                      # [e, db, p]
    ln2do[:, :, 0] = rw_r[0].T
    ln2do[:, :, 1] = rw_r[1].T

    masks = np.zeros((128, 4, 512), f32)
    p_i = np.arange(128)[:, None]
    t_i = np.arange(512)[None, :]
    for j in range(4):
        masks[:, j, :] = (t_i >= 128 * j + p_i)

    sel2b = np.zeros((2, 2, 64), f32)                      # [j, h, m] = (j==h)
    sel2b[0, 0, :] = 1.0
    sel2b[1, 1, :] = 1.0
    sele = np.zeros((2, 2, 128), f32)                      # [j, e, m] = (j==e)
    sele[0, 0, :] = 1.0
    sele[1, 1, :] = 1.0

    def stat_pack(Wsh):  # [128(m), D] -> [128(kp), DB, 128(m)] lhsT layout
        return np.ascontiguousarray(
            Wsh.T.reshape(DB, 128, 128).transpose(1, 0, 2))

    in_maps = []
    for c in range(NCORES):
        hs = slice(128 * c, 128 * c + 128)
        Wq_sh = (Wq * ln1_w[None, :])[hs] * scale_q        # [128, D]
        Wk_sh = (Wk * ln1_w[None, :])[hs]
        Wv_sh = (Wv * ln1_w[None, :])[hs]
        bq = (Wq[hs] @ ln1_b) * scale_q
        bk = Wk[hs] @ ln1_b
        bv = Wv[hs] @ ln1_b
        Wo_sh = Wo[:, hs]                                  # [D, 128]
        wo_pack = np.ascontiguousarray(
            Wo_sh.reshape(DB, 128, 128).transpose(2, 0, 1))  # [i, db, m]

        fs = slice(FFS * c, FFS * c + FFS)
        egt = np.stack([
            np.ascontiguousarray(
                eg[e][fs].T.reshape(DB, 128, NFB, 128).transpose(1, 0, 2, 3))
            for e in range(2)])
        ept = np.stack([
            np.ascontiguousarray(
                ep[e][fs].T.reshape(DB, 128, NFB, 128).transpose(1, 0, 2, 3))
            for e in range(2)])
        eot = np.stack([
            np.ascontiguousarray(
                eo[e][:, fs].T.reshape(NFB, 128, D).transpose(1, 0, 2))
            for e in range(2)])

        in_maps.append({
            "xT": xT,
            "wq": stat_pack(Wq_sh).astype(NPBF16),
            "wk": stat_pack(Wk_sh).astype(NPBF16),
            "wv": stat_pack(Wv_sh).astype(NPBF16),
            "bq": bq.reshape(128, 1).astype(f32),
            "bk": bk.reshape(128, 1).astype(f32),
            "bv": bv.reshape(1, 128).astype(NPBF16),
            "wo": wo_pack.astype(NPBF16),
            "ln2su": ln2su.astype(NPBF16),
            "ln2do": ln2do.astype(NPBF16),
            "scn": scn, "cbias": cbias,
            "masks": masks.astype(NPBF16),
            "sel2b": sel2b.astype(NPBF16),
            "sele": sele.astype(NPBF16),
            "egt": egt.astype(NPBF16),
            "ept": ept.astype(NPBF16),
            "eot": eot.astype(NPBF16),
        })
    return in_maps


def _get_compiled():
    global _COMPILED
    if _COMPILED is None:
        _COMPILED = _build_nc()
    return _COMPILED


def _unshard(results):
    out = np.zeros((NCH, TC, D), np.float32)
    for c in range(NCORES):
        r = np.asarray(results[c]["out_rows"], dtype=np.float32)
        # r[ci, half, i, t] -> out[ci, t, 512*half + 64*c + i]
        for i in range(NCH):
            out[i, :, 64 * c:64 * c + 64] = r[i, 0].T
            out[i, :, 512 + 64 * c:512 + 64 * c + 64] = r[i, 1].T
    return out.reshape(B, T, D)


def kernel(**inputs):
    nc = _get_compiled()
    in_maps = _prep_inputs(inputs)
    res = run_bass_kernel_spmd(nc, in_maps, list(range(NCORES)))
    return _unshard(res.results)
